# revision 5
# baseline (speedup 1.0000x reference)
"""Causal multi-head attention (B=2, T=2048, D=1024, H=16) on 8 trn2 cores.

Sharding: data-parallel over batch (2) x tensor-parallel over heads (4 groups
of 4 heads): core c handles batch c//4, head group c%4. Each core computes
q/k/v projections for its 256 feature columns, causal attention for its 4
heads, and a partial row-parallel output projection. The host sums the 4
partials per batch and adds bo.

Key device-side structure (v2):
- x arrives pre-transposed from the host as [D, T]; no PE transposes.
- k-bias is dropped (softmax is invariant to per-query constants).
- q/k are quantized to fp8e4m3 and attention scores run as DoubleRow
  matmuls (K=64 packed as 32 partitions x 2 k-tiles), 2x PE rate.
- es and v are bf16 for the PV matmuls (1 cycle/row at any width).
- Causal masking is applied post-exp by zeroing the diagonal blocks of es
  with gpsimd affine_select (keeps the Act engine exp-only).
- Pipeline by q-group: stage g = projections for t-group g, then attention
  for q-group g over all heads (all its k-chunks are already projected).
"""

import sys

if "/opt/trn_rl_repo" not in sys.path:
    sys.path.insert(0, "/opt/trn_rl_repo")

import numpy as np

import concourse.bass as bass
import concourse.mybir as mybir
import concourse.tile as tile
from concourse import bacc

F32 = mybir.dt.float32
F32R = mybir.dt.float32r
BF16 = mybir.dt.bfloat16
FP8 = mybir.dt.float8e4
DR = mybir.MatmulPerfMode.DoubleRow
EXP = mybir.ActivationFunctionType.Exp

B, T, D, H, HD = 2, 2048, 1024, 16, 64
SCALE = float(D) ** -0.5
NCORES = 8
HPC = 4  # heads per core
JS = HPC * HD  # 256 feature columns per core
NT = T // 128  # 16 t-chunks
ND = D // 128  # 8 d-chunks
NG = T // 512  # 4 query groups / pipeline stages

_CACHE = {}

FLAGS = {"pv_lag": 3}


def _emit_consts(nc, consts, dram):
    c = {}
    c["wq"] = consts.tile([128, ND, JS], F32R, name="wq_sb")
    c["wk"] = consts.tile([128, ND, JS], F32R, name="wk_sb")
    c["wv"] = consts.tile([128, ND, JS], F32R, name="wv_sb")
    for key in ("wq", "wk", "wv"):
        nc.sync.dma_start(
            out=c[key], in_=dram[key].ap().rearrange("(c p) j -> p c j", p=128)
        )
    c["wo"] = consts.tile([128, 2, D], F32R, name="wo_sb")
    nc.sync.dma_start(
        out=c["wo"], in_=dram["wo"].ap().rearrange("(c p) n -> p c n", p=128)
    )
    c["bq"] = consts.tile([128, 2], F32, name="bq_sb")
    nc.sync.dma_start(out=c["bq"], in_=dram["bq"].ap().rearrange("(c p) -> p c", p=128))
    c["bv"] = consts.tile([128, JS], F32, name="bv_bc")
    nc.gpsimd.dma_start(
        out=c["bv"], in_=bass.AP(tensor=dram["bv"], offset=0, ap=[[0, 128], [1, JS]])
    )
    c["ones"] = consts.tile([128, HPC * NT], BF16, name="ones_sb")
    nc.vector.memset(c["ones"], 1.0)
    return c


def _emit_body(nc, tc, c, persist, dram, rep):
    r = f"r{rep}"
    qT8 = persist["qT8"]
    kT8 = persist["kT8"]
    vv = persist["vv"]
    oT = persist["oT"]
    x_d = dram["x"]
    out_d = dram["out"]
    xr = x_d.ap().rearrange("(c p) t -> p c t", p=128)
    ones_r = c["ones"].rearrange("p (h i o) -> p h i o", h=HPC, o=1)

    with (
        tc.tile_pool(name=f"xT{r}", bufs=1) as xTpool,
        tc.tile_pool(name=f"psb{r}", bufs=2, space="PSUM") as psb,
        tc.tile_pool(name=f"psS{r}", bufs=2, space="PSUM") as psS,
        tc.tile_pool(name=f"psO{r}", bufs=2, space="PSUM") as psO,
        tc.tile_pool(name=f"esb{r}", bufs=6) as esb,
        tc.tile_pool(name=f"nrm{r}", bufs=3) as nrm,
        tc.tile_pool(name=f"osb{r}", bufs=4) as osb,
    ):
        xT = xTpool.tile([128, ND, T], F32R, name=f"xTt{r}")
        # async x loads, one per stage, spread across DMA queues
        dmaq = [nc.sync, nc.gpsimd, nc.sync, nc.gpsimd]
        for g in range(NG):
            dmaq[g].dma_start(
                out=xT[:, :, g * 512:(g + 1) * 512],
                in_=xr[:, :, g * 512:(g + 1) * 512],
            )

        # ---- attention piece machinery (deferred PV for pipelining) ----
        pending = []  # (h, g, es_tile, chunk_pairs)

        def emit_pv_piece():
            h, g, es, cks = pending.pop(0)
            for j, ck in enumerate(cks):
                glo = max(0, ck * 128 - g * 512)
                nc.tensor.matmul(
                    accs[(g, h)][0:HD + 1, glo:512],
                    vv[:, h, ck, :],
                    es[:, j * 512 + glo:(j + 1) * 512],
                    start=(ck == 0),
                    stop=(ck == 4 * g + 3),
                )
            if cks[-1] == 4 * g + 3:
                # head (g, h) complete: normalize into oT
                acc = accs[(g, h)]
                jc, hr = h // 2, (h % 2) * 64
                rc = nrm.tile([1, 512], F32, tag="rc", name=f"rc{r}_{g}_{h}")
                nc.vector.reciprocal(rc, acc[HD:HD + 1, :])
                rb = nrm.tile([64, 512], F32, tag="rb", name=f"rb{r}_{g}_{h}")
                nc.gpsimd.partition_broadcast(rb, rc)
                nc.vector.tensor_mul(
                    oT[hr:hr + 64, jc, g * 512:(g + 1) * 512], acc[0:HD, :], rb
                )

        def emit_qk_piece(h, g, cks):
            ps = psS.tile([128, 1024], F32, name=f"psrow{r}", tag="psrow")
            for j, ck in enumerate(cks):
                qlo = max(0, ck * 128 - g * 512)
                nc.tensor.matmul(
                    ps[:, j * 512 + qlo:(j + 1) * 512],
                    kT8[32 * h:32 * h + 32, :, ck * 128:(ck + 1) * 128],
                    qT8[32 * h:32 * h + 32, :, g * 512 + qlo:(g + 1) * 512],
                    start=True,
                    stop=True,
                    perf_mode=DR,
                    tile_position=(32 * h, 0),
                )
            es = esb.tile([128, 1024], BF16, name=f"es{r}", tag="es")
            lo0 = max(0, cks[0] * 128 - g * 512)
            lo1 = max(0, cks[1] * 128 - g * 512)
            if lo1 == 0:
                nc.scalar.activation(es[:, lo0:1024], ps[:, lo0:1024], EXP, scale=SCALE)
            else:
                nc.scalar.activation(es[:, lo0:512], ps[:, lo0:512], EXP, scale=SCALE)
                nc.scalar.activation(
                    es[:, 512 + lo1:1024], ps[:, 512 + lo1:1024], EXP, scale=SCALE
                )
            # zero the masked upper triangle of diagonal 128-blocks
            for j, ck in enumerate(cks):
                junk = ck * 128 - g * 512
                if 0 <= junk < 512:
                    nc.gpsimd.affine_select(
                        out=es[:, j * 512 + junk:j * 512 + junk + 128],
                        in_=es[:, j * 512 + junk:j * 512 + junk + 128],
                        compare_op=mybir.AluOpType.is_ge,
                        fill=0.0,
                        base=0,
                        pattern=[[1, 128]],
                        channel_multiplier=-1,
                    )
            pending.append((h, g, es, cks))
            if len(pending) > FLAGS["pv_lag"]:
                emit_pv_piece()

        def emit_outproj(i):
            for ng in range(2):
                ps = psb.tile([128, 512], F32, name=f"ps3t{r}", tag="psb")
                for jc in range(2):
                    nc.tensor.matmul(
                        ps,
                        oT[:, jc, i * 128:(i + 1) * 128],
                        c["wo"][:, jc, ng * 512:(ng + 1) * 512],
                        start=(jc == 0),
                        stop=(jc == 1),
                    )
                ob = osb.tile([128, 512], F32, name=f"ob{r}", tag="ob")
                nc.vector.tensor_copy(ob, ps)
                nc.sync.dma_start(
                    out=out_d.ap()[i * 128:(i + 1) * 128, ng * 512:(ng + 1) * 512],
                    in_=ob,
                )

        accs = {}
        for g in range(NG):
            # ---- projections for t-group g ----
            for w_sb, dstT, b_sb in ((c["wk"], kT8, None), (c["wq"], qT8, c["bq"])):
                for jc in range(2):
                    ps = psb.tile([128, 512], F32, name=f"psqk{r}", tag="psb")
                    for dc in range(ND):
                        nc.tensor.matmul(
                            ps,
                            w_sb[:, dc, jc * 128:(jc + 1) * 128],
                            xT[:, dc, g * 512:(g + 1) * 512],
                            start=(dc == 0),
                            stop=(dc == ND - 1),
                        )
                    dst = dstT[:, jc, g * 512:(g + 1) * 512]
                    if b_sb is None:
                        nc.vector.tensor_copy(dst, ps)
                    else:
                        nc.vector.tensor_scalar_add(
                            out=dst, in0=ps, scalar1=b_sb[:, jc:jc + 1]
                        )
            for i in range(4 * g, 4 * g + 4):
                ps = psb.tile([128, 512], F32, name=f"psv{r}", tag="psb")
                for dc in range(ND):
                    nc.tensor.matmul(
                        ps[:, :JS],
                        xT[:, dc, i * 128:(i + 1) * 128],
                        c["wv"][:, dc, :],
                        start=(dc == 0),
                        stop=(dc == ND - 1),
                    )
                nc.vector.tensor_add(
                    out=vv[:, :, i, 0:HD],
                    in0=ps[:, :JS].rearrange("p (h c) -> p h c", h=HPC),
                    in1=c["bv"].rearrange("p (h c) -> p h c", h=HPC),
                )
            nc.vector.tensor_copy(vv[:, :, 4 * g:4 * g + 4, HD:HD + 1],
                                  ones_r[:, :, 4 * g:4 * g + 4, :])
            # ---- output projection for the previous stage ----
            if g > 0:
                for i in range(4 * (g - 1), 4 * g):
                    emit_outproj(i)
            # ---- attention for q-group g, all heads ----
            for h in range(HPC):
                accs[(g, h)] = psO.tile(
                    [128, 512], F32, tag="oacc", name=f"oacc{r}_{g}_{h}"
                )
                for cp in range(2 * g + 2):
                    emit_qk_piece(h, g, (2 * cp, 2 * cp + 1))
            # drain: norms for this stage must be emitted before the next
            # stage's output projection reads oT
            while pending:
                emit_pv_piece()
        for i in range(4 * (NG - 1), NT):
            emit_outproj(i)


def build(reps=1):
    nc = bacc.Bacc("TRN2", target_bir_lowering=False, num_devices=NCORES)
    dram = {
        "x": nc.dram_tensor("x", [D, T], F32R, kind="ExternalInput"),
        "wq": nc.dram_tensor("wq", [D, JS], F32R, kind="ExternalInput"),
        "wk": nc.dram_tensor("wk", [D, JS], F32R, kind="ExternalInput"),
        "wv": nc.dram_tensor("wv", [D, JS], F32R, kind="ExternalInput"),
        "bq": nc.dram_tensor("bq", [JS], F32, kind="ExternalInput"),
        "bv": nc.dram_tensor("bv", [JS], F32, kind="ExternalInput"),
        "wo": nc.dram_tensor("wo", [JS, D], F32R, kind="ExternalInput"),
        "out": nc.dram_tensor("out", [T, D], F32, kind="ExternalOutput"),
    }
    with tile.TileContext(nc) as tc:
        with (
            tc.tile_pool(name="consts", bufs=1) as consts,
            tc.tile_pool(name="persist", bufs=1) as persist_pool,
        ):
            c = _emit_consts(nc, consts, dram)
            persist = {
                "qT8": persist_pool.tile([128, 2, T], FP8, name="qT8"),
                "kT8": persist_pool.tile([128, 2, T], FP8, name="kT8"),
                "vv": persist_pool.tile([128, HPC, NT, HD + 1], BF16, name="vv"),
                "oT": persist_pool.tile([128, 2, T], F32R, name="oT"),
            }
            for rep in range(reps):
                _emit_body(nc, tc, c, persist, dram, rep)
    nc.compile()
    return nc


# permutation of the 256 per-core feature columns for the fp8 DoubleRow
# layout: chunk c (=hd half), head h, lane p  <-  feature h*64 + 32*c + p
_PERM = np.array(
    [h * 64 + 32 * c + p for c in range(2) for h in range(HPC) for p in range(32)]
)


def _in_maps(inputs):
    x = np.asarray(inputs["x"], dtype=np.float32)
    maps = []
    for cc in range(NCORES):
        b, g = cc // HPC, cc % HPC
        js = slice(g * JS, (g + 1) * JS)
        wq = np.asarray(inputs["wq"], np.float32)[:, js][:, _PERM]
        wk = np.asarray(inputs["wk"], np.float32)[:, js][:, _PERM]
        bq = np.asarray(inputs["bq"], np.float32)[js][_PERM]
        maps.append(
            {
                "x": np.ascontiguousarray(x[b].T),
                "wq": np.ascontiguousarray(wq),
                "wk": np.ascontiguousarray(wk),
                "wv": np.ascontiguousarray(np.asarray(inputs["wv"], np.float32)[:, js]),
                "bq": np.ascontiguousarray(bq),
                "bv": np.ascontiguousarray(np.asarray(inputs["bv"], np.float32)[js]),
                "wo": np.ascontiguousarray(np.asarray(inputs["wo"], np.float32)[js, :]),
            }
        )
    return maps


def kernel(**inputs) -> np.ndarray:
    from concourse.bass_utils import run_bass_kernel_spmd

    if "nc" not in _CACHE:
        _CACHE["nc"] = build()
    nc = _CACHE["nc"]
    maps = _in_maps(inputs)
    res = run_bass_kernel_spmd(nc, maps, core_ids=list(range(NCORES)))
    out = np.zeros((B, T, D), dtype=np.float32)
    for cc in range(NCORES):
        out[cc // HPC] += res.results[cc]["out"]
    out += np.asarray(inputs["bo"], np.float32)[None, None, :]
    return out
